# revision 50
# baseline (speedup 1.0000x reference)
"""Multi-headed self-attention (S=2048, D=1024, H=16) on 8 trn2 NeuronCores.

Tensor-parallel over heads (2 heads/core). Restructured for overlap:
 - batched input DMAs (weights first, x per d-tile, w_out last)
 - k/q projections first (8 psum accumulators), then window-pipelined
   attention: chunk c's scores+exp (Act engine) overlap chunk c-1's ctx
   matmuls (PE) with a 1-chunk lag; v-projection and PE-transposes are
   interleaved into window 0's PE slack.
 - engine split: Act = exp only, DVE = copies/normalize, Sync = input
   DMAs + a2a_in writes, GpSimd = collectives/reloads/out DMA.
 - per (head, s-chunk) AllToAll reshards head-split ctx to seq-split for
   the output projection; proj(ci0) is emitted after the last AllToAll
   trigger so it covers the collective wait; warm matmuls hold PE clock.

Self-contained: hardcodes all shapes; host-side prep is limited to
transpose / dtype-cast / slicing of the inputs.
"""

import sys

import numpy as np

if "/opt/trn_rl_repo" not in sys.path:
    sys.path.insert(0, "/opt/trn_rl_repo")

S, D, A, H = 2048, 1024, 1024, 16
NCORES = 8
HPC = H // NCORES            # heads per core = 2
HD = A // H                  # head dim = 64
E = HPC * HD                 # local qkv rows = 128
ND = D // 128                # d tiles = 8
NT = S // 128                # key tiles = 16
LN2 = 0.6931471805599453
EXP_SCALE = LN2 * (HD ** -0.5)   # p = 2^(score/8) = exp(score * ln2/8)

CH = 1024                    # attention s-chunk == AllToAll chunk
NCH = S // CH                # = 2
NMM = 512                    # matmul moving size (hw max 512 elements)

_CACHE = {}


def _build(enable_asserts=False):
    import concourse.bass as bass
    import concourse.tile as tile
    import concourse.mybir as mybir
    from concourse import bacc
    from concourse.masks import make_identity

    f16 = mybir.dt.float16
    f32 = mybir.dt.float32

    nc = bacc.Bacc(
        "TRN2",
        target_bir_lowering=False,
        debug=False,
        enable_asserts=enable_asserts,
        num_devices=NCORES,
    )

    xT = nc.dram_tensor("xT", [D, S], f16, kind="ExternalInput").ap()
    wqT = nc.dram_tensor("wqT", [D, E], f16, kind="ExternalInput").ap()
    wkT = nc.dram_tensor("wkT", [D, E], f16, kind="ExternalInput").ap()
    wvT = nc.dram_tensor("wvT", [D, E], f16, kind="ExternalInput").ap()
    woT = nc.dram_tensor("woT", [A, D], f16, kind="ExternalInput").ap()
    out = nc.dram_tensor("out", [NCH, 128, D], f16, kind="ExternalOutput").ap()

    with tile.TileContext(nc) as tc:
        _body(tc, xT, wqT, wkT, wvT, woT, out, mybir, bass, make_identity)

    nc.compile()
    return nc


def _body(tc, xT, wqT, wkT, wvT, woT, out, mybir, bass, make_identity):
    from contextlib import ExitStack

    nc = tc.nc
    f16 = mybir.dt.float16
    f32 = mybir.dt.float32
    Exp = mybir.ActivationFunctionType.Exp

    ctx_stack = ExitStack()
    persist = ctx_stack.enter_context(tc.tile_pool(name="persist", bufs=1))

    def ptile(shape, dtype, name):
        return persist.tile(shape, dtype, tag=name, name=name)

    xt_sb = ptile([128, ND, S], f16, "xt_sb")        # x.T, d-tile major
    wq_sb = ptile([128, ND, E], f16, "wq_sb")
    wk_sb = ptile([128, ND, E], f16, "wk_sb")
    wv_sb = ptile([128, ND, E], f16, "wv_sb")
    wo_sb = ptile([128, ND, D], f16, "wo_sb")
    qT_sb = ptile([128, S], f16, "qT_sb")            # [2*hd, s]
    kT_sb = ptile([128, S], f16, "kT_sb")
    vT_sb = ptile([128, S], f16, "vT_sb")
    # v' per t-tile: [v_h0 | ones | v_h1 | ones] -> cols [0:65] and [65:130]
    vp_sb = ptile([128, NT, 2 * (HD + 1)], f16, "vp_sb")
    ident_sb = ptile([128, 128], f16, "ident_sb")
    ones_sb = ptile([HD + 1, HD], f16, "ones_sb")
    # normalized ctx.T per head (base partition 0 each)
    ctxn_h = [ptile([HD, S], f16, f"ctxn_h{h}") for h in range(HPC)]
    ctxf_sb = [
        ptile([128, NCORES, 128], f16, f"ctxf_sb{ci}") for ci in range(NCH)
    ]
    dummy_sb = ptile([1, 32], f16, "dummy_sb")
    dummy32a = ptile([1, 32], f32, "dummy32a")
    dummy32b = ptile([1, 32], f32, "dummy32b")

    make_identity(nc, ident_sb[:])
    nc.vector.memset(ones_sb[:], 1.0)
    nc.vector.memset(vp_sb[:, :, HD:HD + 1], 1.0)
    nc.vector.memset(vp_sb[:, :, 2 * HD + 1:2 * HD + 2], 1.0)
    # preload Exp act table + DVE recip uop table during the DMA wait
    nc.scalar.activation(dummy_sb[:], ident_sb[0:1, 0:32], Exp)
    nc.vector.memset(dummy32a[:], 1.0)
    nc.vector.reciprocal_approx_fast(dummy32b[:], dummy32a[:])

    # ---- input loads: kq weights first; x split across two DMA queues
    # (sync even d-tiles, scalar odd) ----
    nc.sync.dma_start(wk_sb[:], wkT.rearrange("(nd p) e -> p nd e", p=128))
    nc.scalar.dma_start(wq_sb[:], wqT.rearrange("(nd p) e -> p nd e", p=128))
    for dt_ in range(ND):
        eng = nc.sync if dt_ % 2 == 0 else nc.scalar
        eng.dma_start(xt_sb[:, dt_, :], xT[dt_ * 128:(dt_ + 1) * 128, :])
    nc.sync.dma_start(wv_sb[:], wvT.rearrange("(nd p) e -> p nd e", p=128))
    nc.sync.dma_start(wo_sb[:], woT.rearrange("(a p) d -> p a d", p=128))

    NKQ = S // NMM            # moving chunks for kq proj (2 @ NMM=1024)

    # ---- k/q projections: 2*NKQ psum accumulators over all 8 banks ----
    with tc.tile_pool(name="kq_ps", bufs=1, space="PSUM") as kq_ps:
        acc = {}
        for wname in ("k", "q"):
            for c in range(NKQ):
                acc[(wname, c)] = kq_ps.tile(
                    [128, NMM], f32, tag=f"a{wname}{c}", name=f"a{wname}{c}"
                )
        # warm the PE p-state while the x DMAs stream (results discarded:
        # the first real matmul resets the bank with start=True)
        for i in range(10):
            nc.tensor.matmul(
                acc[("k", 0)][:, 0:512],
                lhsT=ident_sb[:],
                rhs=qT_sb[:, 0:512],
                start=True,
                stop=True,
            )
        for dt_ in range(ND):
            for wname, wsb in (("k", wk_sb), ("q", wq_sb)):
                for c in range(NKQ):
                    nc.tensor.matmul(
                        acc[(wname, c)][:],
                        lhsT=wsb[:, dt_, :],
                        rhs=xt_sb[:, dt_, c * NMM:(c + 1) * NMM],
                        start=(dt_ == 0),
                        stop=(dt_ == ND - 1),
                    )
        # copies in need-order: the first scores (chunk (h0,ci0), low tt)
        # need k chunk 0 + q chunks 0,1; later k chunks and q 2,3 trail
        nc.vector.tensor_copy(kT_sb[:, 0:512], acc[("k", 0)][:])
        nc.scalar.copy(qT_sb[:, 0:512], acc[("q", 0)][:])
        nc.vector.tensor_copy(qT_sb[:, 512:1024], acc[("q", 1)][:])
        nc.scalar.copy(kT_sb[:, 512:1024], acc[("k", 1)][:])
        nc.vector.tensor_copy(kT_sb[:, 1024:1536], acc[("k", 2)][:])
        nc.scalar.copy(kT_sb[:, 1536:2048], acc[("k", 3)][:])
        nc.vector.tensor_copy(qT_sb[:, 1024:1536], acc[("q", 2)][:])
        nc.scalar.copy(qT_sb[:, 1536:2048], acc[("q", 3)][:])

    # ---- attention: chunks with 1-window lag between scores and ctx ----
    chunks = [(0, 0), (1, 0), (0, 1), (1, 1)]   # (h, ci), ci-outer

    dram = ctx_stack.enter_context(tc.tile_pool(name="dram", bufs=1, space="DRAM"))
    a2a_in = [
        dram.tile([NCORES, 128, 128], f16, name=f"a2a_in{ci}") for ci in range(NCH)
    ]
    a2a_out = [
        dram.tile([NCORES, 128, 128], f16, name=f"a2a_out{ci}") for ci in range(NCH)
    ]

    sc_ps = ctx_stack.enter_context(tc.tile_pool(name="sc_ps", bufs=3, space="PSUM"))
    pt_pool = ctx_stack.enter_context(tc.tile_pool(name="pt_pool", bufs=20))
    misc = ctx_stack.enter_context(tc.tile_pool(name="misc", bufs=2))
    out_pool = ctx_stack.enter_context(tc.tile_pool(name="out_pool", bufs=2))

    pts = {}

    i16 = mybir.dt.int16
    Mul = mybir.AluOpType.mult
    Add = mybir.AluOpType.add
    # p = 2^(score/8): fp16 bit trick t = score*128 + B, int16(t) viewed
    # as fp16 is 2^i*(1 + f - c) ~= 2^(i+f).  B = 15360 - 45 centers the
    # mantissa-interp error; scores*128 stays in (0, 21000) so t > 0.
    B_SCHR = 15315.0

    def emit_score(c, tt):
        # the two 512-col score halves go to separate psum tiles with
        # independent WAR rings: sca is read only by Act (exp), scb only
        # by DVE (bit-trick exp), so neither ring waits on the other
        h, ci = chunks[c]
        hb = h * HD
        sca = sc_ps.tile([128, 512], f32, tag="sca", name="sca")
        scb = sc_ps.tile([128, 512], f32, tag="scb", name="scb")
        for nn, sc in ((0, sca), (1, scb)):
            nc.tensor.matmul(
                sc[:],
                lhsT=kT_sb[hb:hb + HD, tt * 128:(tt + 1) * 128],
                rhs=qT_sb[hb:hb + HD,
                          ci * CH + nn * NMM:ci * CH + (nn + 1) * NMM],
                start=True,
                stop=True,
                tile_position=(hb, 0),
            )
        pt = pt_pool.tile([128, CH], f16, tag="pt", name="pt")
        nc.scalar.activation(pt[:, 0:512], sca[:], Exp, scale=EXP_SCALE)
        # offloaded half on DVE, one instr: int16(score*128 + B) whose
        # bits, read back as f16, are 2^(score/8)
        nc.vector.tensor_scalar(
            pt[:, 512:1024].bitcast(i16), scb[:], 128.0, B_SCHR, Mul, Add
        )
        pts[(c, tt)] = pt

    def emit_ctx(c, tt, ctx):
        h, ci = chunks[c]
        pt = pts.pop((c, tt))
        for nn in range(CH // NMM):
            nc.tensor.matmul(
                ctx[:, nn * NMM:(nn + 1) * NMM],
                lhsT=vp_sb[:, tt, h * (HD + 1):(h + 1) * (HD + 1)],
                rhs=pt[:, nn * NMM:(nn + 1) * NMM],
                start=(tt == 0),
                stop=(tt == NT - 1),
            )

    # window 0 filler worklist: v-proj (sc-major), v copies, transposes
    def v_work(v_ps):
        vacc = [None] * 4

        def mms(s, dt_):
            if dt_ == 0:
                vacc[s] = v_ps.tile([128, 512], f32, tag="v", name=f"vacc{s}")
            nc.tensor.matmul(
                vacc[s][:],
                lhsT=wv_sb[:, dt_, :],
                rhs=xt_sb[:, dt_, s * 512:(s + 1) * 512],
                start=(dt_ == 0),
                stop=(dt_ == ND - 1),
            )

        def vcopy(s):
            nc.vector.tensor_copy(vT_sb[:, s * 512:(s + 1) * 512], vacc[s][:])

        def tr(t):
            tp = v_ps.tile([128, 128], f16, tag="v", name="tp")
            nc.tensor.transpose(
                tp[:], vT_sb[:, t * 128:(t + 1) * 128], ident_sb[:]
            )
            nc.vector.tensor_copy(vp_sb[:, t, 0:HD], tp[:, 0:HD])
            nc.vector.tensor_copy(
                vp_sb[:, t, HD + 1:2 * HD + 1], tp[:, HD:2 * HD]
            )

        # 2 rotating psum slots: pair (s, s+1) accumulates, then copies out
        # and the freed slots take the transposes / the next pair.
        for s0 in (0, 2):
            for dt_ in range(ND):
                yield lambda a=s0, d=dt_: mms(a, d)
                yield lambda a=s0 + 1, d=dt_: mms(a, d)
            yield lambda a=s0: vcopy(a)
            yield lambda a=s0 + 1: vcopy(a)
            for t in range(4 * s0, 4 * s0 + 8):
                yield lambda t=t: tr(t)

    with tc.tile_pool(name="v_ps", bufs=2, space="PSUM") as v_ps:
        work = v_work(v_ps)
        done = False
        for tt in range(NT):
            emit_score(0, tt)
            for _ in range(4 if tt < 4 else 3):
                try:
                    next(work)()
                except StopIteration:
                    done = True
                    break
        while not done:
            try:
                next(work)()
            except StopIteration:
                done = True

    attn_ps = ctx_stack.enter_context(tc.tile_pool(name="attn_ps", bufs=1, space="PSUM"))

    def emit_norm(c):
        h, ci = chunks[c]
        ctx = ctx_tiles[c]
        den = misc.tile([HD + 1, CH], f16, tag="den", name="den")
        for nn in range(2):
            scr = sc_ps.tile([128, 512], f32, tag="sca" if nn == 0 else "scb", name="scr")
            nc.scalar.copy(
                den[HD:HD + 1, nn * 512:(nn + 1) * 512],
                ctx[HD:HD + 1, nn * 512:(nn + 1) * 512],
            )
            nc.tensor.matmul(
                scr[0:HD, :],
                lhsT=ones_sb[HD:HD + 1, :],
                rhs=den[HD:HD + 1, nn * 512:(nn + 1) * 512],
                start=True,
                stop=True,
                tile_position=(HD, 0),
            )
            rbc = misc.tile([HD, 512], f32, tag="rbc", name="rbc")
            nc.vector.reciprocal_approx_fast(rbc[:], scr[0:HD, :])
            nc.vector.tensor_mul(
                ctxn_h[h][:, ci * CH + nn * 512:ci * CH + (nn + 1) * 512],
                ctx[0:HD, nn * 512:(nn + 1) * 512],
                rbc[:],
            )

    def emit_a2a_half(ci, h):
        nc.sync.dma_start(
            a2a_in[ci][:, h * HD:(h + 1) * HD, :].rearrange("r p s -> p r s"),
            ctxn_h[h][:, ci * CH:(ci + 1) * CH].rearrange(
                "p (r s) -> p r s", r=NCORES
            ),
        )

    def emit_a2a(ci):
        # one collective per s-chunk carrying both heads
        nc.gpsimd.collective_compute(
            "AllToAll",
            mybir.AluOpType.bypass,
            replica_groups=[list(range(NCORES))],
            ins=[a2a_in[ci].opt()],
            outs=[a2a_out[ci].opt()],
        )

    def emit_reload(ci):
        # split across two queues so the fragmented (rank-major ->
        # partition-major) descriptor streams run in parallel
        nc.gpsimd.dma_start(
            ctxf_sb[ci][0:HD, :, :],
            a2a_out[ci][:, 0:HD, :].rearrange("r p s -> p r s"),
        )
        nc.sync.dma_start(
            ctxf_sb[ci][HD:2 * HD, :, :],
            a2a_out[ci][:, HD:2 * HD, :].rearrange("r p s -> p r s"),
        )

    def emit_proj(ci):
        # kt outer so each ctxf[:, kt, :] stationary feeds both nn matmuls
        ob = out_pool.tile([128, D], f16, tag="ob", name="ob")
        psa = sc_ps.tile([128, 512], f32, tag="sca", name="proj_psa")
        psb = sc_ps.tile([128, 512], f32, tag="scb", name="proj_psb")
        for kt in range(ND):
            for nn, ps in ((0, psa), (1, psb)):
                nc.tensor.matmul(
                    ps[:],
                    lhsT=ctxf_sb[ci][:, kt, :],
                    rhs=wo_sb[:, kt, nn * 512:(nn + 1) * 512],
                    start=(kt == 0),
                    stop=(kt == ND - 1),
                )
        nc.vector.tensor_copy(ob[:, 0:512], psa[:])
        nc.scalar.copy(ob[:, 512:1024], psb[:])
        nc.gpsimd.dma_start(out[ci], ob[:])

    ctx_tiles = {}

    # windows 1..3: scores(c) + ctx(c-1); window 4: ctx(3) only
    for c in range(1, 4):
        ctx_tiles[c - 1] = attn_ps.tile([HD + 1, CH], f32, tag="ctx", name="ctx", bufs=1)
        for tt in range(NT):
            emit_score(c, tt)
            emit_ctx(c - 1, tt, ctx_tiles[c - 1])
        emit_norm(c - 1)
        emit_a2a_half(0 if c < 3 else 1, chunks[c - 1][0])
        if c == 2:
            emit_a2a(0)
    ctx_tiles[3] = attn_ps.tile([HD + 1, CH], f32, tag="ctx", name="ctx", bufs=1)
    for tt in range(NT):
        emit_ctx(3, tt, ctx_tiles[3])
    emit_norm(3)
    emit_a2a_half(1, 1)
    emit_a2a(1)

    # tail: proj(ci0) covers the last AllToAll wait with real work; warm
    # matmuls hold the PE clock; reload(ci1) gates only proj(ci1).
    def emit_warm(n, rhs):
        for i in range(n):
            warm = attn_ps.tile([HD, 512], f32, tag="ctx", name="warm", bufs=1)
            nc.tensor.matmul(
                warm[:, 0:rhs.free_size()],
                lhsT=ones_sb[HD:HD + 1, :],
                rhs=rhs,
                start=True,
                stop=True,
                tile_position=(HD, 0),
            )

    # blind warms first: nothing reload-gated may sit at the PE queue head
    # while the norm(c3) -> a2a(ci1) chain drains
    emit_warm(6, kT_sb[HD:HD + 1, 0:512])
    emit_reload(0)
    emit_proj(0)
    emit_warm(2, kT_sb[HD:HD + 1, 0:512])
    emit_reload(1)
    # these warms wait on the reload, re-raising the PE clock (needs ~3us
    # of gap-free execution) right before the final projection
    emit_warm(2, ctxf_sb[1][HD:HD + 1, 0:4, :])
    emit_proj(1)

    ctx_stack.close()


def get_nc(enable_asserts=False):
    key = ("nc", enable_asserts)
    if key not in _CACHE:
        _CACHE[key] = _build(enable_asserts)
    return _CACHE[key]


def make_in_maps(x, w_in, w_out):
    x = np.asarray(x, dtype=np.float32)
    w_in = np.asarray(w_in, dtype=np.float32)
    w_out = np.asarray(w_out, dtype=np.float32)
    xT = np.ascontiguousarray(x.T).astype(np.float16)
    w_outT = w_out.T.astype(np.float16)          # [A(e), D]
    in_maps = []
    for c in range(NCORES):
        r0 = c * E
        wq = np.ascontiguousarray(w_in[r0:r0 + E].T).astype(np.float16)
        wk = np.ascontiguousarray(w_in[A + r0:A + r0 + E].T).astype(np.float16)
        wv = np.ascontiguousarray(
            w_in[2 * A + r0:2 * A + r0 + E].T
        ).astype(np.float16)
        in_maps.append(
            {"xT": xT, "wqT": wq, "wkT": wk, "wvT": wv, "woT": w_outT}
        )
    return in_maps


def assemble_out(results):
    """results[c]["out"] is [NCH, 128, D] fp16; strip ci = out rows
    [ci*CH + c*128 : +128]."""
    full = np.empty((S, D), dtype=np.float32)
    for c in range(NCORES):
        o = results[c]["out"]
        for ci in range(NCH):
            r0 = ci * CH + c * 128
            full[r0:r0 + 128] = o[ci].astype(np.float32)
    return full


def kernel(x, w_in, w_out, tgt_len=None, **kwargs):
    from concourse.bass_utils import run_bass_kernel_spmd

    nc = get_nc()
    in_maps = make_in_maps(x, w_in, w_out)
    res = run_bass_kernel_spmd(nc, in_maps, core_ids=list(range(NCORES)))
    return assemble_out(res.results)


# revision 53
# speedup vs baseline: 1.0796x; 1.0796x over previous
"""Multi-headed self-attention (S=2048, D=1024, H=16) on 8 trn2 NeuronCores.

Tensor-parallel over heads (2 heads/core). Restructured for overlap:
 - batched input DMAs (weights first, x per d-tile, w_out last)
 - k/q projections first (8 psum accumulators), then window-pipelined
   attention: chunk c's scores+exp (Act engine) overlap chunk c-1's ctx
   matmuls (PE) with a 1-chunk lag; v-projection and PE-transposes are
   interleaved into window 0's PE slack.
 - engine split: Act = exp only, DVE = copies/normalize, Sync = input
   DMAs + a2a_in writes, GpSimd = collectives/reloads/out DMA.
 - per (head, s-chunk) AllToAll reshards head-split ctx to seq-split for
   the output projection; proj(ci0) is emitted after the last AllToAll
   trigger so it covers the collective wait; warm matmuls hold PE clock.

Self-contained: hardcodes all shapes; host-side prep is limited to
transpose / dtype-cast / slicing of the inputs.
"""

import sys

import numpy as np

if "/opt/trn_rl_repo" not in sys.path:
    sys.path.insert(0, "/opt/trn_rl_repo")

S, D, A, H = 2048, 1024, 1024, 16
NCORES = 8
HPC = H // NCORES            # heads per core = 2
HD = A // H                  # head dim = 64
E = HPC * HD                 # local qkv rows = 128
ND = D // 128                # d tiles = 8
NT = S // 128                # key tiles = 16
LN2 = 0.6931471805599453
EXP_SCALE = LN2 * (HD ** -0.5)   # p = 2^(score/8) = exp(score * ln2/8)

CH = 1024                    # attention s-chunk == AllToAll chunk
NCH = S // CH                # = 2
NMM = 512                    # matmul moving size (hw max 512 elements)

_CACHE = {}


def _build(enable_asserts=False):
    import concourse.bass as bass
    import concourse.tile as tile
    import concourse.mybir as mybir
    from concourse import bacc
    from concourse.masks import make_identity

    f16 = mybir.dt.float16
    f32 = mybir.dt.float32

    nc = bacc.Bacc(
        "TRN2",
        target_bir_lowering=False,
        debug=False,
        enable_asserts=enable_asserts,
        num_devices=NCORES,
    )

    xT = nc.dram_tensor("xT", [D, S], f16, kind="ExternalInput").ap()
    wqT = nc.dram_tensor("wqT", [D, E], f16, kind="ExternalInput").ap()
    wkT = nc.dram_tensor("wkT", [D, E], f16, kind="ExternalInput").ap()
    wvT = nc.dram_tensor("wvT", [D, E], f16, kind="ExternalInput").ap()
    woT = nc.dram_tensor("woT", [A, D], f16, kind="ExternalInput").ap()
    out = nc.dram_tensor("out", [NCH, 128, D], f16, kind="ExternalOutput").ap()

    with tile.TileContext(nc) as tc:
        _body(tc, xT, wqT, wkT, wvT, woT, out, mybir, bass, make_identity)

    nc.compile()
    return nc


def _body(tc, xT, wqT, wkT, wvT, woT, out, mybir, bass, make_identity):
    from contextlib import ExitStack

    nc = tc.nc
    f16 = mybir.dt.float16
    f32 = mybir.dt.float32
    Exp = mybir.ActivationFunctionType.Exp

    ctx_stack = ExitStack()
    persist = ctx_stack.enter_context(tc.tile_pool(name="persist", bufs=1))

    def ptile(shape, dtype, name):
        return persist.tile(shape, dtype, tag=name, name=name)

    xt_sb = ptile([128, ND, S], f16, "xt_sb")        # x.T, d-tile major
    wq_sb = ptile([128, ND, E], f16, "wq_sb")
    wk_sb = ptile([128, ND, E], f16, "wk_sb")
    wv_sb = ptile([128, ND, E], f16, "wv_sb")
    wo_sb = ptile([128, ND, D], f16, "wo_sb")
    qT_sb = ptile([128, S], f16, "qT_sb")            # [2*hd, s]
    kT_sb = ptile([128, S], f16, "kT_sb")
    vT_sb = ptile([128, S], f16, "vT_sb")
    # v' per t-tile: [v_h0 | ones | v_h1 | ones] -> cols [0:65] and [65:130]
    vp_sb = ptile([128, NT, 2 * (HD + 1)], f16, "vp_sb")
    ident_sb = ptile([128, 128], f16, "ident_sb")
    ones_sb = ptile([HD + 1, HD], f16, "ones_sb")
    # normalized ctx.T per head (base partition 0 each)
    ctxn_h = [ptile([HD, S], f16, f"ctxn_h{h}") for h in range(HPC)]
    ctxf_sb = [
        ptile([128, NCORES, 128], f16, f"ctxf_sb{ci}") for ci in range(NCH)
    ]
    dummy_sb = ptile([1, 32], f16, "dummy_sb")
    dummy32a = ptile([1, 32], f32, "dummy32a")
    dummy32b = ptile([1, 32], f32, "dummy32b")

    make_identity(nc, ident_sb[:])
    nc.vector.memset(ones_sb[:], 1.0)
    nc.vector.memset(vp_sb[:, :, HD:HD + 1], 1.0)
    nc.vector.memset(vp_sb[:, :, 2 * HD + 1:2 * HD + 2], 1.0)
    # preload Exp act table + DVE recip uop table during the DMA wait
    nc.scalar.activation(dummy_sb[:], ident_sb[0:1, 0:32], Exp)
    nc.vector.memset(dummy32a[:], 1.0)
    nc.vector.reciprocal_approx_fast(dummy32b[:], dummy32a[:])

    # ---- input loads: kq weights first; x split across two DMA queues
    # (sync even d-tiles, scalar odd) ----
    nc.sync.dma_start(wk_sb[:], wkT.rearrange("(nd p) e -> p nd e", p=128))
    nc.scalar.dma_start(wq_sb[:], wqT.rearrange("(nd p) e -> p nd e", p=128))
    for dt_ in range(ND):
        eng = nc.sync if dt_ % 2 == 0 else nc.scalar
        eng.dma_start(xt_sb[:, dt_, :], xT[dt_ * 128:(dt_ + 1) * 128, :])
    nc.sync.dma_start(wv_sb[:], wvT.rearrange("(nd p) e -> p nd e", p=128))
    nc.sync.dma_start(wo_sb[:], woT.rearrange("(a p) d -> p a d", p=128))

    NKQ = S // NMM            # moving chunks for kq proj (2 @ NMM=1024)

    # ---- k/q projections: 2*NKQ psum accumulators over all 8 banks ----
    with tc.tile_pool(name="kq_ps", bufs=1, space="PSUM") as kq_ps:
        acc = {}
        for wname in ("k", "q"):
            for c in range(NKQ):
                acc[(wname, c)] = kq_ps.tile(
                    [128, NMM], f32, tag=f"a{wname}{c}", name=f"a{wname}{c}"
                )
        # warm the PE p-state while the x DMAs stream (results discarded:
        # the first real matmul resets the bank with start=True)
        for i in range(10):
            nc.tensor.matmul(
                acc[("k", 0)][:, 0:512],
                lhsT=ident_sb[:],
                rhs=qT_sb[:, 0:512],
                start=True,
                stop=True,
            )
        for dt_ in range(ND):
            for wname, wsb in (("k", wk_sb), ("q", wq_sb)):
                for c in range(NKQ):
                    nc.tensor.matmul(
                        acc[(wname, c)][:],
                        lhsT=wsb[:, dt_, :],
                        rhs=xt_sb[:, dt_, c * NMM:(c + 1) * NMM],
                        start=(dt_ == 0),
                        stop=(dt_ == ND - 1),
                    )
        # copies in need-order: the first scores (chunk (h0,ci0), low tt)
        # need k chunk 0 + q chunks 0,1; later k chunks and q 2,3 trail
        nc.vector.tensor_copy(kT_sb[:, 0:512], acc[("k", 0)][:])
        nc.scalar.copy(qT_sb[:, 0:512], acc[("q", 0)][:])
        nc.vector.tensor_copy(qT_sb[:, 512:1024], acc[("q", 1)][:])
        nc.scalar.copy(kT_sb[:, 512:1024], acc[("k", 1)][:])
        nc.vector.tensor_copy(kT_sb[:, 1024:1536], acc[("k", 2)][:])
        nc.scalar.copy(kT_sb[:, 1536:2048], acc[("k", 3)][:])
        nc.vector.tensor_copy(qT_sb[:, 1024:1536], acc[("q", 2)][:])
        nc.scalar.copy(qT_sb[:, 1536:2048], acc[("q", 3)][:])

    # ---- attention: chunks with 1-window lag between scores and ctx ----
    chunks = [(0, 0), (1, 0), (0, 1), (1, 1)]   # (h, ci), ci-outer

    dram = ctx_stack.enter_context(tc.tile_pool(name="dram", bufs=1, space="DRAM"))
    a2a_in = [
        dram.tile([NCORES, 128, 128], f16, name=f"a2a_in{ci}") for ci in range(NCH)
    ]
    a2a_out = [
        dram.tile([NCORES, 128, 128], f16, name=f"a2a_out{ci}") for ci in range(NCH)
    ]

    sc_ps = ctx_stack.enter_context(tc.tile_pool(name="sc_ps", bufs=3, space="PSUM"))
    pt_pool = ctx_stack.enter_context(tc.tile_pool(name="pt_pool", bufs=20))
    misc = ctx_stack.enter_context(tc.tile_pool(name="misc", bufs=2))
    out_pool = ctx_stack.enter_context(tc.tile_pool(name="out_pool", bufs=2))

    pts = {}

    i16 = mybir.dt.int16
    Mul = mybir.AluOpType.mult
    Add = mybir.AluOpType.add
    # p = 2^(score/8): fp16 bit trick t = score*128 + B, int16(t) viewed
    # as fp16 is 2^i*(1 + f - c) ~= 2^(i+f).  B = 15360 - 45 centers the
    # mantissa-interp error; scores*128 stays in (0, 21000) so t > 0.
    B_SCHR = 15315.0

    def emit_score(c, tt):
        # the two 512-col score halves go to separate psum tiles with
        # independent WAR rings: sca is read only by Act (exp), scb only
        # by DVE (bit-trick exp), so neither ring waits on the other
        h, ci = chunks[c]
        hb = h * HD
        sca = sc_ps.tile([128, 512], f32, tag="sca", name="sca")
        scb = sc_ps.tile([128, 512], f32, tag="scb", name="scb", bufs=2)
        for nn, sc in ((0, sca), (1, scb)):
            nc.tensor.matmul(
                sc[:],
                lhsT=kT_sb[hb:hb + HD, tt * 128:(tt + 1) * 128],
                rhs=qT_sb[hb:hb + HD,
                          ci * CH + nn * NMM:ci * CH + (nn + 1) * NMM],
                start=True,
                stop=True,
                tile_position=(hb, 0),
            )
        pt = pt_pool.tile([128, CH], f16, tag="pt", name="pt")
        nc.scalar.activation(pt[:, 0:512], sca[:], Exp, scale=EXP_SCALE)
        # offloaded half on DVE, one instr: int16(score*128 + B) whose
        # bits, read back as f16, are 2^(score/8)
        nc.vector.tensor_scalar(
            pt[:, 512:1024].bitcast(i16), scb[:], 128.0, B_SCHR, Mul, Add
        )
        pts[(c, tt)] = pt

    def emit_ctx(c, tt, ctx):
        h, ci = chunks[c]
        pt = pts.pop((c, tt))
        for nn in range(CH // NMM):
            nc.tensor.matmul(
                ctx[:, nn * NMM:(nn + 1) * NMM],
                lhsT=vp_sb[:, tt, h * (HD + 1):(h + 1) * (HD + 1)],
                rhs=pt[:, nn * NMM:(nn + 1) * NMM],
                start=(tt == 0),
                stop=(tt == NT - 1),
            )

    # window 0 filler worklist: v-proj (sc-major), v copies, transposes
    def v_work(v_ps):
        vacc = [None] * 4

        def mms(s, dt_):
            if dt_ == 0:
                vacc[s] = v_ps.tile([128, 512], f32, tag="v", name=f"vacc{s}")
            nc.tensor.matmul(
                vacc[s][:],
                lhsT=wv_sb[:, dt_, :],
                rhs=xt_sb[:, dt_, s * 512:(s + 1) * 512],
                start=(dt_ == 0),
                stop=(dt_ == ND - 1),
            )

        def vcopy(s):
            nc.vector.tensor_copy(vT_sb[:, s * 512:(s + 1) * 512], vacc[s][:])

        def tr(t):
            tp = v_ps.tile([128, 128], f16, tag="v", name="tp")
            nc.tensor.transpose(
                tp[:], vT_sb[:, t * 128:(t + 1) * 128], ident_sb[:]
            )
            nc.vector.tensor_copy(vp_sb[:, t, 0:HD], tp[:, 0:HD])
            nc.vector.tensor_copy(
                vp_sb[:, t, HD + 1:2 * HD + 1], tp[:, HD:2 * HD]
            )

        # 2 rotating psum slots: pair (s, s+1) accumulates, then copies out
        # and the freed slots take the transposes / the next pair.
        for s0 in (0, 2):
            for dt_ in range(ND):
                yield lambda a=s0, d=dt_: mms(a, d)
                yield lambda a=s0 + 1, d=dt_: mms(a, d)
            yield lambda a=s0: vcopy(a)
            yield lambda a=s0 + 1: vcopy(a)
            for t in range(4 * s0, 4 * s0 + 8):
                yield lambda t=t: tr(t)

    with tc.tile_pool(name="v_ps", bufs=2, space="PSUM") as v_ps:
        work = v_work(v_ps)
        done = False
        for tt in range(NT):
            emit_score(0, tt)
            for _ in range(4 if tt < 4 else 3):
                try:
                    next(work)()
                except StopIteration:
                    done = True
                    break
        while not done:
            try:
                next(work)()
            except StopIteration:
                done = True

    attn_ps = ctx_stack.enter_context(tc.tile_pool(name="attn_ps", bufs=1, space="PSUM"))

    def emit_norm(c):
        h, ci = chunks[c]
        ctx = ctx_tiles[c]
        den = misc.tile([HD + 1, CH], f16, tag="den", name="den")
        for nn in range(2):
            # dedicated psum bank: the norm chain must never rotate
            # through the score rings (it would stall the next window)
            scr = attn_ps.tile([128, 512], f32, tag="scr", name="scr", bufs=1)
            nc.scalar.copy(
                den[HD:HD + 1, nn * 512:(nn + 1) * 512],
                ctx[HD:HD + 1, nn * 512:(nn + 1) * 512],
            )
            nc.tensor.matmul(
                scr[0:HD, :],
                lhsT=ones_sb[HD:HD + 1, :],
                rhs=den[HD:HD + 1, nn * 512:(nn + 1) * 512],
                start=True,
                stop=True,
                tile_position=(HD, 0),
            )
            rbc = misc.tile([HD, 512], f32, tag="rbc", name="rbc")
            nc.vector.reciprocal_approx_fast(rbc[:], scr[0:HD, :])
            nc.vector.tensor_mul(
                ctxn_h[h][:, ci * CH + nn * 512:ci * CH + (nn + 1) * 512],
                ctx[0:HD, nn * 512:(nn + 1) * 512],
                rbc[:],
            )

    def emit_a2a_half(ci, h):
        nc.sync.dma_start(
            a2a_in[ci][:, h * HD:(h + 1) * HD, :].rearrange("r p s -> p r s"),
            ctxn_h[h][:, ci * CH:(ci + 1) * CH].rearrange(
                "p (r s) -> p r s", r=NCORES
            ),
        )

    def emit_a2a(ci):
        # one collective per s-chunk carrying both heads
        nc.gpsimd.collective_compute(
            "AllToAll",
            mybir.AluOpType.bypass,
            replica_groups=[list(range(NCORES))],
            ins=[a2a_in[ci].opt()],
            outs=[a2a_out[ci].opt()],
        )

    def emit_reload(ci):
        # split across two queues so the fragmented (rank-major ->
        # partition-major) descriptor streams run in parallel
        nc.gpsimd.dma_start(
            ctxf_sb[ci][0:HD, :, :],
            a2a_out[ci][:, 0:HD, :].rearrange("r p s -> p r s"),
        )
        nc.sync.dma_start(
            ctxf_sb[ci][HD:2 * HD, :, :],
            a2a_out[ci][:, HD:2 * HD, :].rearrange("r p s -> p r s"),
        )

    def emit_proj(ci):
        # kt outer so each ctxf[:, kt, :] stationary feeds both nn matmuls
        ob = out_pool.tile([128, D], f16, tag="ob", name="ob")
        psa = sc_ps.tile([128, 512], f32, tag="sca", name="proj_psa")
        psb = sc_ps.tile([128, 512], f32, tag="scb", name="proj_psb", bufs=2)
        for kt in range(ND):
            for nn, ps in ((0, psa), (1, psb)):
                nc.tensor.matmul(
                    ps[:],
                    lhsT=ctxf_sb[ci][:, kt, :],
                    rhs=wo_sb[:, kt, nn * 512:(nn + 1) * 512],
                    start=(kt == 0),
                    stop=(kt == ND - 1),
                )
        nc.vector.tensor_copy(ob[:, 0:512], psa[:])
        nc.scalar.copy(ob[:, 512:1024], psb[:])
        nc.gpsimd.dma_start(out[ci], ob[:])

    ctx_tiles = {}

    # windows 1..3: scores(c) + ctx(c-1); window 4: ctx(3) only
    for c in range(1, 4):
        ctx_tiles[c - 1] = attn_ps.tile([HD + 1, CH], f32, tag="ctx", name="ctx", bufs=1)
        for tt in range(NT):
            emit_score(c, tt)
            emit_ctx(c - 1, tt, ctx_tiles[c - 1])
        emit_norm(c - 1)
        emit_a2a_half(0 if c < 3 else 1, chunks[c - 1][0])
        if c == 2:
            emit_a2a(0)
    ctx_tiles[3] = attn_ps.tile([HD + 1, CH], f32, tag="ctx", name="ctx", bufs=1)
    for tt in range(NT):
        emit_ctx(3, tt, ctx_tiles[3])
    emit_norm(3)
    emit_a2a_half(1, 1)
    emit_a2a(1)

    # tail: proj(ci0) covers the last AllToAll wait with real work; warm
    # matmuls hold the PE clock; reload(ci1) gates only proj(ci1).
    def emit_warm(n, rhs):
        for i in range(n):
            warm = attn_ps.tile([HD, 512], f32, tag="ctx", name="warm", bufs=1)
            nc.tensor.matmul(
                warm[:, 0:rhs.free_size()],
                lhsT=ones_sb[HD:HD + 1, :],
                rhs=rhs,
                start=True,
                stop=True,
                tile_position=(HD, 0),
            )

    # blind warms first: nothing reload-gated may sit at the PE queue head
    # while the norm(c3) -> a2a(ci1) chain drains
    emit_warm(6, kT_sb[HD:HD + 1, 0:512])
    emit_reload(0)
    emit_proj(0)
    emit_warm(2, kT_sb[HD:HD + 1, 0:512])
    emit_reload(1)
    # these warms wait on the reload, re-raising the PE clock (needs ~3us
    # of gap-free execution) right before the final projection
    emit_warm(2, ctxf_sb[1][HD:HD + 1, 0:4, :])
    emit_proj(1)

    ctx_stack.close()


def get_nc(enable_asserts=False):
    key = ("nc", enable_asserts)
    if key not in _CACHE:
        _CACHE[key] = _build(enable_asserts)
    return _CACHE[key]


def make_in_maps(x, w_in, w_out):
    x = np.asarray(x, dtype=np.float32)
    w_in = np.asarray(w_in, dtype=np.float32)
    w_out = np.asarray(w_out, dtype=np.float32)
    xT = np.ascontiguousarray(x.T).astype(np.float16)
    w_outT = w_out.T.astype(np.float16)          # [A(e), D]
    in_maps = []
    for c in range(NCORES):
        r0 = c * E
        wq = np.ascontiguousarray(w_in[r0:r0 + E].T).astype(np.float16)
        wk = np.ascontiguousarray(w_in[A + r0:A + r0 + E].T).astype(np.float16)
        wv = np.ascontiguousarray(
            w_in[2 * A + r0:2 * A + r0 + E].T
        ).astype(np.float16)
        in_maps.append(
            {"xT": xT, "wqT": wq, "wkT": wk, "wvT": wv, "woT": w_outT}
        )
    return in_maps


def assemble_out(results):
    """results[c]["out"] is [NCH, 128, D] fp16; strip ci = out rows
    [ci*CH + c*128 : +128]."""
    full = np.empty((S, D), dtype=np.float32)
    for c in range(NCORES):
        o = results[c]["out"]
        for ci in range(NCH):
            r0 = ci * CH + c * 128
            full[r0:r0 + 128] = o[ci].astype(np.float32)
    return full


def kernel(x, w_in, w_out, tgt_len=None, **kwargs):
    from concourse.bass_utils import run_bass_kernel_spmd

    nc = get_nc()
    in_maps = make_in_maps(x, w_in, w_out)
    res = run_bass_kernel_spmd(nc, in_maps, core_ids=list(range(NCORES)))
    return assemble_out(res.results)


# revision 54
# speedup vs baseline: 1.1689x; 1.0827x over previous
"""Multi-headed self-attention (S=2048, D=1024, H=16) on 8 trn2 NeuronCores.

Tensor-parallel over heads (2 heads/core). Restructured for overlap:
 - batched input DMAs (weights first, x per d-tile, w_out last)
 - k/q projections first (8 psum accumulators), then window-pipelined
   attention: chunk c's scores+exp (Act engine) overlap chunk c-1's ctx
   matmuls (PE) with a 1-chunk lag; v-projection and PE-transposes are
   interleaved into window 0's PE slack.
 - engine split: Act = exp only, DVE = copies/normalize, Sync = input
   DMAs + a2a_in writes, GpSimd = collectives/reloads/out DMA.
 - per (head, s-chunk) AllToAll reshards head-split ctx to seq-split for
   the output projection; proj(ci0) is emitted after the last AllToAll
   trigger so it covers the collective wait; warm matmuls hold PE clock.

Self-contained: hardcodes all shapes; host-side prep is limited to
transpose / dtype-cast / slicing of the inputs.
"""

import sys

import numpy as np

if "/opt/trn_rl_repo" not in sys.path:
    sys.path.insert(0, "/opt/trn_rl_repo")

S, D, A, H = 2048, 1024, 1024, 16
NCORES = 8
HPC = H // NCORES            # heads per core = 2
HD = A // H                  # head dim = 64
E = HPC * HD                 # local qkv rows = 128
ND = D // 128                # d tiles = 8
NT = S // 128                # key tiles = 16
LN2 = 0.6931471805599453
EXP_SCALE = LN2 * (HD ** -0.5)   # p = 2^(score/8) = exp(score * ln2/8)

CH = 1024                    # attention s-chunk == AllToAll chunk
NCH = S // CH                # = 2
NMM = 512                    # matmul moving size (hw max 512 elements)

_CACHE = {}


def _build(enable_asserts=False):
    import concourse.bass as bass
    import concourse.tile as tile
    import concourse.mybir as mybir
    from concourse import bacc
    from concourse.masks import make_identity

    f16 = mybir.dt.float16
    f32 = mybir.dt.float32

    nc = bacc.Bacc(
        "TRN2",
        target_bir_lowering=False,
        debug=False,
        enable_asserts=enable_asserts,
        num_devices=NCORES,
    )

    xT = nc.dram_tensor("xT", [D, S], f16, kind="ExternalInput").ap()
    wqT = nc.dram_tensor("wqT", [D, E], f16, kind="ExternalInput").ap()
    wkT = nc.dram_tensor("wkT", [D, E], f16, kind="ExternalInput").ap()
    wvT = nc.dram_tensor("wvT", [D, E], f16, kind="ExternalInput").ap()
    woT = nc.dram_tensor("woT", [A, D], f16, kind="ExternalInput").ap()
    out = nc.dram_tensor("out", [NCH, 128, D], f16, kind="ExternalOutput").ap()

    with tile.TileContext(nc) as tc:
        _body(tc, xT, wqT, wkT, wvT, woT, out, mybir, bass, make_identity)

    nc.compile()
    return nc


def _body(tc, xT, wqT, wkT, wvT, woT, out, mybir, bass, make_identity):
    from contextlib import ExitStack

    nc = tc.nc
    f16 = mybir.dt.float16
    f32 = mybir.dt.float32
    Exp = mybir.ActivationFunctionType.Exp

    ctx_stack = ExitStack()
    persist = ctx_stack.enter_context(tc.tile_pool(name="persist", bufs=1))

    def ptile(shape, dtype, name):
        return persist.tile(shape, dtype, tag=name, name=name)

    xt_sb = ptile([128, ND, S], f16, "xt_sb")        # x.T, d-tile major
    wq_sb = ptile([128, ND, E], f16, "wq_sb")
    wk_sb = ptile([128, ND, E], f16, "wk_sb")
    wv_sb = ptile([128, ND, E], f16, "wv_sb")
    wo_sb = ptile([128, ND, D], f16, "wo_sb")
    qT_sb = ptile([128, S], f16, "qT_sb")            # [2*hd, s]
    kT_sb = ptile([128, S], f16, "kT_sb")
    vT_sb = ptile([128, S], f16, "vT_sb")
    # v' per t-tile: [v_h0 | ones | v_h1 | ones] -> cols [0:65] and [65:130]
    vp_sb = ptile([128, NT, 2 * (HD + 1)], f16, "vp_sb")
    ident_sb = ptile([128, 128], f16, "ident_sb")
    ones_sb = ptile([HD + 1, HD], f16, "ones_sb")
    # normalized ctx.T per head (base partition 0 each)
    ctxn_h = [ptile([HD, S], f16, f"ctxn_h{h}") for h in range(HPC)]
    ctxf_sb = [
        ptile([128, NCORES, 128], f16, f"ctxf_sb{ci}") for ci in range(NCH)
    ]
    dummy_sb = ptile([1, 32], f16, "dummy_sb")
    dummy32a = ptile([1, 32], f32, "dummy32a")
    dummy32b = ptile([1, 32], f32, "dummy32b")

    make_identity(nc, ident_sb[:])
    nc.vector.memset(ones_sb[:], 1.0)
    nc.vector.memset(vp_sb[:, :, HD:HD + 1], 1.0)
    nc.vector.memset(vp_sb[:, :, 2 * HD + 1:2 * HD + 2], 1.0)
    # preload Exp act table + DVE recip uop table during the DMA wait
    nc.scalar.activation(dummy_sb[:], ident_sb[0:1, 0:32], Exp)
    nc.vector.memset(dummy32a[:], 1.0)
    nc.vector.reciprocal_approx_fast(dummy32b[:], dummy32a[:])

    # ---- input loads: kq weights first; x split across two DMA queues
    # (sync even d-tiles, scalar odd) ----
    nc.sync.dma_start(wk_sb[:], wkT.rearrange("(nd p) e -> p nd e", p=128))
    nc.scalar.dma_start(wq_sb[:], wqT.rearrange("(nd p) e -> p nd e", p=128))
    for dt_ in range(ND):
        eng = nc.sync if dt_ % 2 == 0 else nc.scalar
        eng.dma_start(xt_sb[:, dt_, :], xT[dt_ * 128:(dt_ + 1) * 128, :])
    nc.sync.dma_start(wv_sb[:], wvT.rearrange("(nd p) e -> p nd e", p=128))
    nc.sync.dma_start(wo_sb[:], woT.rearrange("(a p) d -> p a d", p=128))

    NKQ = S // NMM            # moving chunks for kq proj (2 @ NMM=1024)

    # ---- k/q projections: 2*NKQ psum accumulators over all 8 banks ----
    with tc.tile_pool(name="kq_ps", bufs=1, space="PSUM") as kq_ps:
        acc = {}
        for wname in ("k", "q"):
            for c in range(NKQ):
                acc[(wname, c)] = kq_ps.tile(
                    [128, NMM], f32, tag=f"a{wname}{c}", name=f"a{wname}{c}"
                )
        # warm the PE p-state while the x DMAs stream (results discarded:
        # the first real matmul resets the bank with start=True)
        for i in range(10):
            nc.tensor.matmul(
                acc[("k", 0)][:, 0:512],
                lhsT=ident_sb[:],
                rhs=qT_sb[:, 0:512],
                start=True,
                stop=True,
            )
        for dt_ in range(ND):
            for wname, wsb in (("k", wk_sb), ("q", wq_sb)):
                for c in range(NKQ):
                    nc.tensor.matmul(
                        acc[(wname, c)][:],
                        lhsT=wsb[:, dt_, :],
                        rhs=xt_sb[:, dt_, c * NMM:(c + 1) * NMM],
                        start=(dt_ == 0),
                        stop=(dt_ == ND - 1),
                    )
        # copies in need-order: the first scores (chunk (h0,ci0), low tt)
        # need k chunk 0 + q chunks 0,1; later k chunks and q 2,3 trail
        nc.vector.tensor_copy(kT_sb[:, 0:512], acc[("k", 0)][:])
        nc.scalar.copy(qT_sb[:, 0:512], acc[("q", 0)][:])
        nc.vector.tensor_copy(qT_sb[:, 512:1024], acc[("q", 1)][:])
        nc.scalar.copy(kT_sb[:, 512:1024], acc[("k", 1)][:])
        nc.vector.tensor_copy(kT_sb[:, 1024:1536], acc[("k", 2)][:])
        nc.scalar.copy(kT_sb[:, 1536:2048], acc[("k", 3)][:])
        nc.vector.tensor_copy(qT_sb[:, 1024:1536], acc[("q", 2)][:])
        nc.scalar.copy(qT_sb[:, 1536:2048], acc[("q", 3)][:])

    # ---- attention: chunks with 1-window lag between scores and ctx ----
    chunks = [(0, 0), (1, 0), (0, 1), (1, 1)]   # (h, ci), ci-outer

    dram = ctx_stack.enter_context(tc.tile_pool(name="dram", bufs=1, space="DRAM"))
    a2a_in = [
        dram.tile([NCORES, 128, 128], f16, name=f"a2a_in{ci}") for ci in range(NCH)
    ]
    a2a_out = [
        dram.tile([NCORES, 128, 128], f16, name=f"a2a_out{ci}") for ci in range(NCH)
    ]

    sc_ps = ctx_stack.enter_context(tc.tile_pool(name="sc_ps", bufs=3, space="PSUM"))
    pt_pool = ctx_stack.enter_context(tc.tile_pool(name="pt_pool", bufs=20))
    misc = ctx_stack.enter_context(tc.tile_pool(name="misc", bufs=2))
    out_pool = ctx_stack.enter_context(tc.tile_pool(name="out_pool", bufs=2))

    pts = {}

    i16 = mybir.dt.int16
    Mul = mybir.AluOpType.mult
    Add = mybir.AluOpType.add
    # p = 2^(score/8): fp16 bit trick t = score*128 + B, int16(t) viewed
    # as fp16 is 2^i*(1 + f - c) ~= 2^(i+f).  B = 15360 - 45 centers the
    # mantissa-interp error; scores*128 stays in (0, 21000) so t > 0.
    B_SCHR = 15315.0

    def emit_score(c, tt):
        # the two 512-col score halves go to separate psum tiles with
        # independent WAR rings: sca is read only by Act (exp), scb only
        # by DVE (bit-trick exp), so neither ring waits on the other
        h, ci = chunks[c]
        hb = h * HD
        sca = sc_ps.tile([128, 512], f32, tag="sca", name="sca")
        scb = sc_ps.tile([128, 512], f32, tag="scb", name="scb", bufs=2)
        for nn, sc in ((0, sca), (1, scb)):
            nc.tensor.matmul(
                sc[:],
                lhsT=kT_sb[hb:hb + HD, tt * 128:(tt + 1) * 128],
                rhs=qT_sb[hb:hb + HD,
                          ci * CH + nn * NMM:ci * CH + (nn + 1) * NMM],
                start=True,
                stop=True,
                tile_position=(hb, 0),
            )
        pt = pt_pool.tile([128, CH], f16, tag="pt", name="pt")
        nc.scalar.activation(pt[:, 0:512], sca[:], Exp, scale=EXP_SCALE)
        # offloaded half on DVE, one instr: int16(score*128 + B) whose
        # bits, read back as f16, are 2^(score/8)
        nc.vector.tensor_scalar(
            pt[:, 512:1024].bitcast(i16), scb[:], 128.0, B_SCHR, Mul, Add
        )
        pts[(c, tt)] = pt

    def emit_ctx(c, tt, ctx):
        h, ci = chunks[c]
        pt = pts.pop((c, tt))
        for nn in range(CH // NMM):
            nc.tensor.matmul(
                ctx[:, nn * NMM:(nn + 1) * NMM],
                lhsT=vp_sb[:, tt, h * (HD + 1):(h + 1) * (HD + 1)],
                rhs=pt[:, nn * NMM:(nn + 1) * NMM],
                start=(tt == 0),
                stop=(tt == NT - 1),
            )

    # window 0 filler worklist: v-proj (sc-major), v copies, transposes
    def v_work(v_ps):
        vacc = [None] * 4

        def mms(s, dt_):
            if dt_ == 0:
                vacc[s] = v_ps.tile([128, 512], f32, tag="v", name=f"vacc{s}")
            nc.tensor.matmul(
                vacc[s][:],
                lhsT=wv_sb[:, dt_, :],
                rhs=xt_sb[:, dt_, s * 512:(s + 1) * 512],
                start=(dt_ == 0),
                stop=(dt_ == ND - 1),
            )

        def vcopy(s):
            nc.vector.tensor_copy(vT_sb[:, s * 512:(s + 1) * 512], vacc[s][:])

        def tr(t):
            tp = v_ps.tile([128, 128], f16, tag="v", name="tp")
            nc.tensor.transpose(
                tp[:], vT_sb[:, t * 128:(t + 1) * 128], ident_sb[:]
            )
            nc.vector.tensor_copy(vp_sb[:, t, 0:HD], tp[:, 0:HD])
            nc.vector.tensor_copy(
                vp_sb[:, t, HD + 1:2 * HD + 1], tp[:, HD:2 * HD]
            )

        # 2 rotating psum slots: pair (s, s+1) accumulates, then copies out
        # and the freed slots take the transposes / the next pair.
        for s0 in (0, 2):
            for dt_ in range(ND):
                yield lambda a=s0, d=dt_: mms(a, d)
                yield lambda a=s0 + 1, d=dt_: mms(a, d)
            yield lambda a=s0: vcopy(a)
            yield lambda a=s0 + 1: vcopy(a)
            for t in range(4 * s0, 4 * s0 + 8):
                yield lambda t=t: tr(t)

    with tc.tile_pool(name="v_ps", bufs=2, space="PSUM") as v_ps:
        work = v_work(v_ps)
        done = False
        for tt in range(NT):
            emit_score(0, tt)
            for _ in range(4 if tt < 4 else 3):
                try:
                    next(work)()
                except StopIteration:
                    done = True
                    break
        while not done:
            try:
                next(work)()
            except StopIteration:
                done = True

    attn_ps = ctx_stack.enter_context(tc.tile_pool(name="attn_ps", bufs=1, space="PSUM"))

    def emit_norm(c):
        h, ci = chunks[c]
        ctx = ctx_tiles[c]
        # one fast Scalar copy frees the ctx psum tile (~1.1us) so the next
        # window's ctx matmul does not stall behind the whole norm chain;
        # everything below works from the sbuf copy
        ctxc = misc.tile([HD + 1, CH], f16, tag="ctxc", name="ctxc")
        nc.scalar.copy(ctxc[:], ctx[:])
        for nn in range(2):
            # dedicated psum bank: the norm chain must never rotate
            # through the score rings (it would stall the next window)
            scr = attn_ps.tile([128, 512], f32, tag="scr", name="scr", bufs=1)
            nc.tensor.matmul(
                scr[0:HD, :],
                lhsT=ones_sb[HD:HD + 1, :],
                rhs=ctxc[HD:HD + 1, nn * 512:(nn + 1) * 512],
                start=True,
                stop=True,
                tile_position=(HD, 0),
            )
            rbc = misc.tile([HD, 512], f32, tag="rbc", name="rbc")
            nc.vector.reciprocal_approx_fast(rbc[:], scr[0:HD, :])
            nc.vector.tensor_mul(
                ctxn_h[h][:, ci * CH + nn * 512:ci * CH + (nn + 1) * 512],
                ctxc[0:HD, nn * 512:(nn + 1) * 512],
                rbc[:],
            )

    def emit_a2a_half(ci, h):
        nc.sync.dma_start(
            a2a_in[ci][:, h * HD:(h + 1) * HD, :].rearrange("r p s -> p r s"),
            ctxn_h[h][:, ci * CH:(ci + 1) * CH].rearrange(
                "p (r s) -> p r s", r=NCORES
            ),
        )

    def emit_a2a(ci):
        # one collective per s-chunk carrying both heads
        nc.gpsimd.collective_compute(
            "AllToAll",
            mybir.AluOpType.bypass,
            replica_groups=[list(range(NCORES))],
            ins=[a2a_in[ci].opt()],
            outs=[a2a_out[ci].opt()],
        )

    def emit_reload(ci):
        # split across two queues so the fragmented (rank-major ->
        # partition-major) descriptor streams run in parallel
        nc.gpsimd.dma_start(
            ctxf_sb[ci][0:HD, :, :],
            a2a_out[ci][:, 0:HD, :].rearrange("r p s -> p r s"),
        )
        nc.sync.dma_start(
            ctxf_sb[ci][HD:2 * HD, :, :],
            a2a_out[ci][:, HD:2 * HD, :].rearrange("r p s -> p r s"),
        )

    def emit_proj(ci):
        # kt outer so each ctxf[:, kt, :] stationary feeds both nn matmuls
        ob = out_pool.tile([128, D], f16, tag="ob", name="ob")
        psa = sc_ps.tile([128, 512], f32, tag="sca", name="proj_psa")
        psb = sc_ps.tile([128, 512], f32, tag="scb", name="proj_psb", bufs=2)
        for kt in range(ND):
            for nn, ps in ((0, psa), (1, psb)):
                nc.tensor.matmul(
                    ps[:],
                    lhsT=ctxf_sb[ci][:, kt, :],
                    rhs=wo_sb[:, kt, nn * 512:(nn + 1) * 512],
                    start=(kt == 0),
                    stop=(kt == ND - 1),
                )
        nc.vector.tensor_copy(ob[:, 0:512], psa[:])
        nc.scalar.copy(ob[:, 512:1024], psb[:])
        nc.gpsimd.dma_start(out[ci], ob[:])

    ctx_tiles = {}

    # windows 1..3: scores(c) + ctx(c-1); window 4: ctx(3) only
    for c in range(1, 4):
        ctx_tiles[c - 1] = attn_ps.tile([HD + 1, CH], f32, tag="ctx", name="ctx", bufs=1)
        for tt in range(NT):
            emit_score(c, tt)
            emit_ctx(c - 1, tt, ctx_tiles[c - 1])
        emit_norm(c - 1)
        emit_a2a_half(0 if c < 3 else 1, chunks[c - 1][0])
        if c == 2:
            emit_a2a(0)
    ctx_tiles[3] = attn_ps.tile([HD + 1, CH], f32, tag="ctx", name="ctx", bufs=1)
    for tt in range(NT):
        emit_ctx(3, tt, ctx_tiles[3])
    emit_norm(3)
    emit_a2a_half(1, 1)
    emit_a2a(1)

    # tail: proj(ci0) covers the last AllToAll wait with real work; warm
    # matmuls hold the PE clock; reload(ci1) gates only proj(ci1).
    def emit_warm(n, rhs):
        for i in range(n):
            warm = attn_ps.tile([HD, 512], f32, tag="ctx", name="warm", bufs=1)
            nc.tensor.matmul(
                warm[:, 0:rhs.free_size()],
                lhsT=ones_sb[HD:HD + 1, :],
                rhs=rhs,
                start=True,
                stop=True,
                tile_position=(HD, 0),
            )

    # blind warms first: nothing reload-gated may sit at the PE queue head
    # while the norm(c3) -> a2a(ci1) chain drains
    emit_warm(6, kT_sb[HD:HD + 1, 0:512])
    emit_reload(0)
    emit_proj(0)
    emit_warm(2, kT_sb[HD:HD + 1, 0:512])
    emit_reload(1)
    # these warms wait on the reload, re-raising the PE clock (needs ~3us
    # of gap-free execution) right before the final projection
    emit_warm(2, ctxf_sb[1][HD:HD + 1, 0:4, :])
    emit_proj(1)

    ctx_stack.close()


def get_nc(enable_asserts=False):
    key = ("nc", enable_asserts)
    if key not in _CACHE:
        _CACHE[key] = _build(enable_asserts)
    return _CACHE[key]


def make_in_maps(x, w_in, w_out):
    x = np.asarray(x, dtype=np.float32)
    w_in = np.asarray(w_in, dtype=np.float32)
    w_out = np.asarray(w_out, dtype=np.float32)
    xT = np.ascontiguousarray(x.T).astype(np.float16)
    w_outT = w_out.T.astype(np.float16)          # [A(e), D]
    in_maps = []
    for c in range(NCORES):
        r0 = c * E
        wq = np.ascontiguousarray(w_in[r0:r0 + E].T).astype(np.float16)
        wk = np.ascontiguousarray(w_in[A + r0:A + r0 + E].T).astype(np.float16)
        wv = np.ascontiguousarray(
            w_in[2 * A + r0:2 * A + r0 + E].T
        ).astype(np.float16)
        in_maps.append(
            {"xT": xT, "wqT": wq, "wkT": wk, "wvT": wv, "woT": w_outT}
        )
    return in_maps


def assemble_out(results):
    """results[c]["out"] is [NCH, 128, D] fp16; strip ci = out rows
    [ci*CH + c*128 : +128]."""
    full = np.empty((S, D), dtype=np.float32)
    for c in range(NCORES):
        o = results[c]["out"]
        for ci in range(NCH):
            r0 = ci * CH + c * 128
            full[r0:r0 + 128] = o[ci].astype(np.float32)
    return full


def kernel(x, w_in, w_out, tgt_len=None, **kwargs):
    from concourse.bass_utils import run_bass_kernel_spmd

    nc = get_nc()
    in_maps = make_in_maps(x, w_in, w_out)
    res = run_bass_kernel_spmd(nc, in_maps, core_ids=list(range(NCORES)))
    return assemble_out(res.results)


# revision 55
# speedup vs baseline: 1.1764x; 1.0064x over previous
"""Multi-headed self-attention (S=2048, D=1024, H=16) on 8 trn2 NeuronCores.

Tensor-parallel over heads (2 heads/core). Restructured for overlap:
 - batched input DMAs (weights first, x per d-tile, w_out last)
 - k/q projections first (8 psum accumulators), then window-pipelined
   attention: chunk c's scores+exp (Act engine) overlap chunk c-1's ctx
   matmuls (PE) with a 1-chunk lag; v-projection and PE-transposes are
   interleaved into window 0's PE slack.
 - engine split: Act = exp only, DVE = copies/normalize, Sync = input
   DMAs + a2a_in writes, GpSimd = collectives/reloads/out DMA.
 - per (head, s-chunk) AllToAll reshards head-split ctx to seq-split for
   the output projection; proj(ci0) is emitted after the last AllToAll
   trigger so it covers the collective wait; warm matmuls hold PE clock.

Self-contained: hardcodes all shapes; host-side prep is limited to
transpose / dtype-cast / slicing of the inputs.
"""

import sys

import numpy as np

if "/opt/trn_rl_repo" not in sys.path:
    sys.path.insert(0, "/opt/trn_rl_repo")

S, D, A, H = 2048, 1024, 1024, 16
NCORES = 8
HPC = H // NCORES            # heads per core = 2
HD = A // H                  # head dim = 64
E = HPC * HD                 # local qkv rows = 128
ND = D // 128                # d tiles = 8
NT = S // 128                # key tiles = 16
LN2 = 0.6931471805599453
EXP_SCALE = LN2 * (HD ** -0.5)   # p = 2^(score/8) = exp(score * ln2/8)

CH = 1024                    # attention s-chunk == AllToAll chunk
NCH = S // CH                # = 2
NMM = 512                    # matmul moving size (hw max 512 elements)

_CACHE = {}


def _build(enable_asserts=False):
    import concourse.bass as bass
    import concourse.tile as tile
    import concourse.mybir as mybir
    from concourse import bacc
    from concourse.masks import make_identity

    f16 = mybir.dt.float16
    f32 = mybir.dt.float32

    nc = bacc.Bacc(
        "TRN2",
        target_bir_lowering=False,
        debug=False,
        enable_asserts=enable_asserts,
        num_devices=NCORES,
    )

    xT = nc.dram_tensor("xT", [D, S], f16, kind="ExternalInput").ap()
    wqT = nc.dram_tensor("wqT", [D, E], f16, kind="ExternalInput").ap()
    wkT = nc.dram_tensor("wkT", [D, E], f16, kind="ExternalInput").ap()
    wvT = nc.dram_tensor("wvT", [D, E], f16, kind="ExternalInput").ap()
    woT = nc.dram_tensor("woT", [A, D], f16, kind="ExternalInput").ap()
    out = nc.dram_tensor("out", [NCH, 128, D], f16, kind="ExternalOutput").ap()

    with tile.TileContext(nc) as tc:
        _body(tc, xT, wqT, wkT, wvT, woT, out, mybir, bass, make_identity)

    nc.compile()
    return nc


def _body(tc, xT, wqT, wkT, wvT, woT, out, mybir, bass, make_identity):
    from contextlib import ExitStack

    nc = tc.nc
    f16 = mybir.dt.float16
    f32 = mybir.dt.float32
    Exp = mybir.ActivationFunctionType.Exp

    ctx_stack = ExitStack()
    persist = ctx_stack.enter_context(tc.tile_pool(name="persist", bufs=1))

    def ptile(shape, dtype, name):
        return persist.tile(shape, dtype, tag=name, name=name)

    xt_sb = ptile([128, ND, S], f16, "xt_sb")        # x.T, d-tile major
    wq_sb = ptile([128, ND, E], f16, "wq_sb")
    wk_sb = ptile([128, ND, E], f16, "wk_sb")
    wv_sb = ptile([128, ND, E], f16, "wv_sb")
    wo_sb = ptile([128, ND, D], f16, "wo_sb")
    qT_sb = ptile([128, S], f16, "qT_sb")            # [2*hd, s]
    kT_sb = ptile([128, S], f16, "kT_sb")
    vT_sb = ptile([128, S], f16, "vT_sb")
    # v' per t-tile: [v_h0 | ones | v_h1 | ones] -> cols [0:65] and [65:130]
    vp_sb = ptile([128, NT, 2 * (HD + 1)], f16, "vp_sb")
    ident_sb = ptile([128, 128], f16, "ident_sb")
    ones_sb = ptile([HD + 1, HD], f16, "ones_sb")
    # normalized ctx.T per head (base partition 0 each)
    ctxn_h = [ptile([HD, S], f16, f"ctxn_h{h}") for h in range(HPC)]
    ctxf_sb = [
        ptile([128, NCORES, 128], f16, f"ctxf_sb{ci}") for ci in range(NCH)
    ]
    dummy_sb = ptile([1, 32], f16, "dummy_sb")
    dummy32a = ptile([1, 32], f32, "dummy32a")
    dummy32b = ptile([1, 32], f32, "dummy32b")

    make_identity(nc, ident_sb[:])
    nc.vector.memset(ones_sb[:], 1.0)
    nc.vector.memset(vp_sb[:, :, HD:HD + 1], 1.0)
    nc.vector.memset(vp_sb[:, :, 2 * HD + 1:2 * HD + 2], 1.0)
    # preload Exp act table + DVE recip uop table during the DMA wait
    nc.scalar.activation(dummy_sb[:], ident_sb[0:1, 0:32], Exp)
    nc.vector.memset(dummy32a[:], 1.0)
    nc.vector.reciprocal_approx_fast(dummy32b[:], dummy32a[:])

    # ---- input loads: kq weights first; x split across two DMA queues
    # (sync even d-tiles, scalar odd) ----
    nc.sync.dma_start(wk_sb[:], wkT.rearrange("(nd p) e -> p nd e", p=128))
    nc.scalar.dma_start(wq_sb[:], wqT.rearrange("(nd p) e -> p nd e", p=128))
    for dt_ in range(ND):
        eng = nc.sync if dt_ % 2 == 0 else nc.scalar
        for hf in range(2):
            eng.dma_start(
                xt_sb[:, dt_, hf * 1024:(hf + 1) * 1024],
                xT[dt_ * 128:(dt_ + 1) * 128, hf * 1024:(hf + 1) * 1024],
            )
    nc.sync.dma_start(wv_sb[:], wvT.rearrange("(nd p) e -> p nd e", p=128))
    nc.sync.dma_start(wo_sb[:], woT.rearrange("(a p) d -> p a d", p=128))

    NKQ = S // NMM            # moving chunks for kq proj (2 @ NMM=1024)

    # ---- k/q projections: 2*NKQ psum accumulators over all 8 banks ----
    with tc.tile_pool(name="kq_ps", bufs=1, space="PSUM") as kq_ps:
        acc = {}
        for wname in ("k", "q"):
            for c in range(NKQ):
                acc[(wname, c)] = kq_ps.tile(
                    [128, NMM], f32, tag=f"a{wname}{c}", name=f"a{wname}{c}"
                )
        # warm the PE p-state while the x DMAs stream (results discarded:
        # the first real matmul resets the bank with start=True)
        for i in range(10):
            nc.tensor.matmul(
                acc[("k", 0)][:, 0:512],
                lhsT=ident_sb[:],
                rhs=qT_sb[:, 0:512],
                start=True,
                stop=True,
            )
        for dt_ in range(ND):
            for wname, wsb in (("k", wk_sb), ("q", wq_sb)):
                for c in range(NKQ):
                    nc.tensor.matmul(
                        acc[(wname, c)][:],
                        lhsT=wsb[:, dt_, :],
                        rhs=xt_sb[:, dt_, c * NMM:(c + 1) * NMM],
                        start=(dt_ == 0),
                        stop=(dt_ == ND - 1),
                    )
        # copies in need-order: the first scores (chunk (h0,ci0), low tt)
        # need k chunk 0 + q chunks 0,1; later k chunks and q 2,3 trail
        nc.vector.tensor_copy(kT_sb[:, 0:512], acc[("k", 0)][:])
        nc.scalar.copy(qT_sb[:, 0:512], acc[("q", 0)][:])
        nc.vector.tensor_copy(qT_sb[:, 512:1024], acc[("q", 1)][:])
        nc.scalar.copy(kT_sb[:, 512:1024], acc[("k", 1)][:])
        nc.vector.tensor_copy(kT_sb[:, 1024:1536], acc[("k", 2)][:])
        nc.scalar.copy(kT_sb[:, 1536:2048], acc[("k", 3)][:])
        nc.vector.tensor_copy(qT_sb[:, 1024:1536], acc[("q", 2)][:])
        nc.scalar.copy(qT_sb[:, 1536:2048], acc[("q", 3)][:])

    # ---- attention: chunks with 1-window lag between scores and ctx ----
    chunks = [(0, 0), (1, 0), (0, 1), (1, 1)]   # (h, ci), ci-outer

    dram = ctx_stack.enter_context(tc.tile_pool(name="dram", bufs=1, space="DRAM"))
    a2a_in = [
        dram.tile([NCORES, 128, 128], f16, name=f"a2a_in{ci}") for ci in range(NCH)
    ]
    a2a_out = [
        dram.tile([NCORES, 128, 128], f16, name=f"a2a_out{ci}") for ci in range(NCH)
    ]

    sc_ps = ctx_stack.enter_context(tc.tile_pool(name="sc_ps", bufs=3, space="PSUM"))
    pt_pool = ctx_stack.enter_context(tc.tile_pool(name="pt_pool", bufs=20))
    misc = ctx_stack.enter_context(tc.tile_pool(name="misc", bufs=2))
    out_pool = ctx_stack.enter_context(tc.tile_pool(name="out_pool", bufs=2))

    pts = {}

    i16 = mybir.dt.int16
    Mul = mybir.AluOpType.mult
    Add = mybir.AluOpType.add
    # p = 2^(score/8): fp16 bit trick t = score*128 + B, int16(t) viewed
    # as fp16 is 2^i*(1 + f - c) ~= 2^(i+f).  B = 15360 - 45 centers the
    # mantissa-interp error; scores*128 stays in (0, 21000) so t > 0.
    B_SCHR = 15315.0

    def emit_score(c, tt):
        # the two 512-col score halves go to separate psum tiles with
        # independent WAR rings: sca is read only by Act (exp), scb only
        # by DVE (bit-trick exp), so neither ring waits on the other
        h, ci = chunks[c]
        hb = h * HD
        sca = sc_ps.tile([128, 512], f32, tag="sca", name="sca")
        scb = sc_ps.tile([128, 512], f32, tag="scb", name="scb", bufs=2)
        for nn, sc in ((0, sca), (1, scb)):
            nc.tensor.matmul(
                sc[:],
                lhsT=kT_sb[hb:hb + HD, tt * 128:(tt + 1) * 128],
                rhs=qT_sb[hb:hb + HD,
                          ci * CH + nn * NMM:ci * CH + (nn + 1) * NMM],
                start=True,
                stop=True,
                tile_position=(hb, 0),
            )
        pt = pt_pool.tile([128, CH], f16, tag="pt", name="pt")
        nc.scalar.activation(pt[:, 0:512], sca[:], Exp, scale=EXP_SCALE)
        # offloaded half on DVE, one instr: int16(score*128 + B) whose
        # bits, read back as f16, are 2^(score/8)
        nc.vector.tensor_scalar(
            pt[:, 512:1024].bitcast(i16), scb[:], 128.0, B_SCHR, Mul, Add
        )
        pts[(c, tt)] = pt

    def emit_ctx(c, tt, ctx):
        h, ci = chunks[c]
        pt = pts.pop((c, tt))
        for nn in range(CH // NMM):
            nc.tensor.matmul(
                ctx[:, nn * NMM:(nn + 1) * NMM],
                lhsT=vp_sb[:, tt, h * (HD + 1):(h + 1) * (HD + 1)],
                rhs=pt[:, nn * NMM:(nn + 1) * NMM],
                start=(tt == 0),
                stop=(tt == NT - 1),
            )

    # window 0 filler worklist: v-proj (sc-major), v copies, transposes
    def v_work(v_ps):
        vacc = [None] * 4

        def mms(s, dt_):
            if dt_ == 0:
                vacc[s] = v_ps.tile([128, 512], f32, tag="v", name=f"vacc{s}")
            nc.tensor.matmul(
                vacc[s][:],
                lhsT=wv_sb[:, dt_, :],
                rhs=xt_sb[:, dt_, s * 512:(s + 1) * 512],
                start=(dt_ == 0),
                stop=(dt_ == ND - 1),
            )

        def vcopy(s):
            nc.vector.tensor_copy(vT_sb[:, s * 512:(s + 1) * 512], vacc[s][:])

        def tr(t):
            tp = v_ps.tile([128, 128], f16, tag="v", name="tp")
            nc.tensor.transpose(
                tp[:], vT_sb[:, t * 128:(t + 1) * 128], ident_sb[:]
            )
            nc.vector.tensor_copy(vp_sb[:, t, 0:HD], tp[:, 0:HD])
            nc.vector.tensor_copy(
                vp_sb[:, t, HD + 1:2 * HD + 1], tp[:, HD:2 * HD]
            )

        # 2 rotating psum slots: pair (s, s+1) accumulates, then copies out
        # and the freed slots take the transposes / the next pair.
        for s0 in (0, 2):
            for dt_ in range(ND):
                yield lambda a=s0, d=dt_: mms(a, d)
                yield lambda a=s0 + 1, d=dt_: mms(a, d)
            yield lambda a=s0: vcopy(a)
            yield lambda a=s0 + 1: vcopy(a)
            for t in range(4 * s0, 4 * s0 + 8):
                yield lambda t=t: tr(t)

    with tc.tile_pool(name="v_ps", bufs=2, space="PSUM") as v_ps:
        work = v_work(v_ps)
        done = False
        for tt in range(NT):
            emit_score(0, tt)
            for _ in range(4 if tt < 4 else 3):
                try:
                    next(work)()
                except StopIteration:
                    done = True
                    break
        while not done:
            try:
                next(work)()
            except StopIteration:
                done = True

    attn_ps = ctx_stack.enter_context(tc.tile_pool(name="attn_ps", bufs=1, space="PSUM"))

    def emit_norm(c):
        h, ci = chunks[c]
        ctx = ctx_tiles[c]
        # one fast Scalar copy frees the ctx psum tile (~1.1us) so the next
        # window's ctx matmul does not stall behind the whole norm chain;
        # everything below works from the sbuf copy
        ctxc = misc.tile([HD + 1, CH], f16, tag="ctxc", name="ctxc")
        nc.scalar.copy(ctxc[:], ctx[:])
        for nn in range(2):
            # dedicated psum bank: the norm chain must never rotate
            # through the score rings (it would stall the next window)
            scr = attn_ps.tile([128, 512], f32, tag="scr", name="scr", bufs=1)
            nc.tensor.matmul(
                scr[0:HD, :],
                lhsT=ones_sb[HD:HD + 1, :],
                rhs=ctxc[HD:HD + 1, nn * 512:(nn + 1) * 512],
                start=True,
                stop=True,
                tile_position=(HD, 0),
            )
            rbc = misc.tile([HD, 512], f32, tag="rbc", name="rbc")
            nc.vector.reciprocal_approx_fast(rbc[:], scr[0:HD, :])
            nc.vector.tensor_mul(
                ctxn_h[h][:, ci * CH + nn * 512:ci * CH + (nn + 1) * 512],
                ctxc[0:HD, nn * 512:(nn + 1) * 512],
                rbc[:],
            )

    def emit_a2a_half(ci, h):
        nc.sync.dma_start(
            a2a_in[ci][:, h * HD:(h + 1) * HD, :].rearrange("r p s -> p r s"),
            ctxn_h[h][:, ci * CH:(ci + 1) * CH].rearrange(
                "p (r s) -> p r s", r=NCORES
            ),
        )

    def emit_a2a(ci):
        # one collective per s-chunk carrying both heads
        nc.gpsimd.collective_compute(
            "AllToAll",
            mybir.AluOpType.bypass,
            replica_groups=[list(range(NCORES))],
            ins=[a2a_in[ci].opt()],
            outs=[a2a_out[ci].opt()],
        )

    def emit_reload(ci):
        # split across two queues so the fragmented (rank-major ->
        # partition-major) descriptor streams run in parallel
        nc.gpsimd.dma_start(
            ctxf_sb[ci][0:HD, :, :],
            a2a_out[ci][:, 0:HD, :].rearrange("r p s -> p r s"),
        )
        nc.sync.dma_start(
            ctxf_sb[ci][HD:2 * HD, :, :],
            a2a_out[ci][:, HD:2 * HD, :].rearrange("r p s -> p r s"),
        )

    def emit_proj(ci):
        # kt outer so each ctxf[:, kt, :] stationary feeds both nn matmuls
        ob = out_pool.tile([128, D], f16, tag="ob", name="ob")
        psa = sc_ps.tile([128, 512], f32, tag="sca", name="proj_psa")
        psb = sc_ps.tile([128, 512], f32, tag="scb", name="proj_psb", bufs=2)
        for kt in range(ND):
            for nn, ps in ((0, psa), (1, psb)):
                nc.tensor.matmul(
                    ps[:],
                    lhsT=ctxf_sb[ci][:, kt, :],
                    rhs=wo_sb[:, kt, nn * 512:(nn + 1) * 512],
                    start=(kt == 0),
                    stop=(kt == ND - 1),
                )
        nc.vector.tensor_copy(ob[:, 0:512], psa[:])
        nc.scalar.copy(ob[:, 512:1024], psb[:])
        nc.gpsimd.dma_start(out[ci], ob[:])

    ctx_tiles = {}

    # windows 1..3: scores(c) + ctx(c-1); window 4: ctx(3) only
    for c in range(1, 4):
        ctx_tiles[c - 1] = attn_ps.tile([HD + 1, CH], f32, tag="ctx", name="ctx", bufs=1)
        for tt in range(NT):
            emit_score(c, tt)
            emit_ctx(c - 1, tt, ctx_tiles[c - 1])
        emit_norm(c - 1)
        emit_a2a_half(0 if c < 3 else 1, chunks[c - 1][0])
        if c == 2:
            emit_a2a(0)
    ctx_tiles[3] = attn_ps.tile([HD + 1, CH], f32, tag="ctx", name="ctx", bufs=1)
    for tt in range(NT):
        emit_ctx(3, tt, ctx_tiles[3])
    emit_norm(3)
    emit_a2a_half(1, 1)
    emit_a2a(1)

    # tail: proj(ci0) covers the last AllToAll wait with real work; warm
    # matmuls hold the PE clock; reload(ci1) gates only proj(ci1).
    def emit_warm(n, rhs):
        for i in range(n):
            warm = attn_ps.tile([HD, 512], f32, tag="ctx", name="warm", bufs=1)
            nc.tensor.matmul(
                warm[:, 0:rhs.free_size()],
                lhsT=ones_sb[HD:HD + 1, :],
                rhs=rhs,
                start=True,
                stop=True,
                tile_position=(HD, 0),
            )

    # blind warms first: nothing reload-gated may sit at the PE queue head
    # while the norm(c3) -> a2a(ci1) chain drains
    emit_warm(6, kT_sb[HD:HD + 1, 0:512])
    emit_reload(0)
    emit_proj(0)
    emit_warm(2, kT_sb[HD:HD + 1, 0:512])
    emit_reload(1)
    # these warms wait on the reload, re-raising the PE clock (needs ~3us
    # of gap-free execution) right before the final projection
    emit_warm(2, ctxf_sb[1][HD:HD + 1, 0:4, :])
    emit_proj(1)

    ctx_stack.close()


def get_nc(enable_asserts=False):
    key = ("nc", enable_asserts)
    if key not in _CACHE:
        _CACHE[key] = _build(enable_asserts)
    return _CACHE[key]


def make_in_maps(x, w_in, w_out):
    x = np.asarray(x, dtype=np.float32)
    w_in = np.asarray(w_in, dtype=np.float32)
    w_out = np.asarray(w_out, dtype=np.float32)
    xT = np.ascontiguousarray(x.T).astype(np.float16)
    w_outT = w_out.T.astype(np.float16)          # [A(e), D]
    in_maps = []
    for c in range(NCORES):
        r0 = c * E
        wq = np.ascontiguousarray(w_in[r0:r0 + E].T).astype(np.float16)
        wk = np.ascontiguousarray(w_in[A + r0:A + r0 + E].T).astype(np.float16)
        wv = np.ascontiguousarray(
            w_in[2 * A + r0:2 * A + r0 + E].T
        ).astype(np.float16)
        in_maps.append(
            {"xT": xT, "wqT": wq, "wkT": wk, "wvT": wv, "woT": w_outT}
        )
    return in_maps


def assemble_out(results):
    """results[c]["out"] is [NCH, 128, D] fp16; strip ci = out rows
    [ci*CH + c*128 : +128]."""
    full = np.empty((S, D), dtype=np.float32)
    for c in range(NCORES):
        o = results[c]["out"]
        for ci in range(NCH):
            r0 = ci * CH + c * 128
            full[r0:r0 + 128] = o[ci].astype(np.float32)
    return full


def kernel(x, w_in, w_out, tgt_len=None, **kwargs):
    from concourse.bass_utils import run_bass_kernel_spmd

    nc = get_nc()
    in_maps = make_in_maps(x, w_in, w_out)
    res = run_bass_kernel_spmd(nc, in_maps, core_ids=list(range(NCORES)))
    return assemble_out(res.results)
